# revision 23
# baseline (speedup 1.0000x reference)
"""BiDAF-style bidirectional attention kernel for Trainium2 (8 NeuronCores).

Full inputs:
  context      (64, 2048, 256) f32
  question     (64, 64, 256)   f32
  w_question   (256,) f32
  w_context    (256,) f32
  w_multiple   (256,) f32
Outputs (tuple):
  c2q (64, 2048, 256) f32
  q2c (64, 1, 256)    f32

Sharding: data-parallel over batch; each of the 8 cores handles 8 batch
elements with the weight vectors replicated.  No cross-core communication.

Math per batch b:
  sim[i,j] = (ctx[i]*w_m)@q[j] + c_w[i] + q_w[j]      (2048, 64)
  c2q = softmax_j(sim) @ q                             (2048, 256)
  q2c = softmax_i(max_j sim) @ ctx                     (1, 256)

Device mapping (per batch, C split into 16 chunks of 128 rows):
  - ctx chunks are transposed on the PE (packed 4-per-PSUM-bank, evacuated
    by the scalar engine) so the E-contraction matmul can run.
  - mm1: lhsT = ctxT chunk [e,c], rhs = qm_aug [e, 65] where cols 0:64 are
    (q*w_m)T and col 64 is w_c -> tri chunk [c, 65] with c_w in col 64.
  - softmax over j via one fused tensor_tensor_reduce (adds q_w broadcast,
    scales by -1, min-reduces -> -rowmax), then Exp on the scalar engine.
  - mm2: lhsT = eT chunk (PE-transposed exp values) [j, c], rhs = question
    [j, e]; the 1/rowsum normalization is fused into the PSUM->SBUF
    evacuation (tensor_scalar_mul).
  - q2c: row maxes (+c_w) -> global softmax over the 2048 rows (PE transpose
    + reduce for the cross-partition max/sum), 32 N=1 matmuls against the
    natural-layout ctx chunks, scaled on evacuation.
"""

import numpy as np

import concourse.bass as bass
import concourse.mybir as mybir
import concourse.tile as tile
from concourse.bass_utils import run_bass_kernel_spmd
from concourse.masks import make_identity

F32 = mybir.dt.float32
ALU = mybir.AluOpType
ACTF = mybir.ActivationFunctionType

B, C, Q, E = 64, 2048, 64, 256
NCORES = 8
BPC = B // NCORES  # batches per core
CCH = C // 128     # 16 c-chunks
ECH = E // 128     # 2 e-chunks


def _emit(tc, nbatches=BPC, stop_after=None):
    def _stop(stage):
        if stop_after is None:
            return False
        order = ["prep", "trctx", "mm1", "soft", "tre", "mm2", "q2c"]
        return order.index(stage) > order.index(stop_after)

    nc = tc.nc

    ctx_d = nc.dram_tensor("context", [BPC, C, E], F32, kind="ExternalInput").ap()
    q_d = nc.dram_tensor("question", [BPC, Q, E], F32, kind="ExternalInput").ap()
    wq_d = nc.dram_tensor("w_question", [E], F32, kind="ExternalInput").ap()
    wc_d = nc.dram_tensor("w_context", [E], F32, kind="ExternalInput").ap()
    wm_d = nc.dram_tensor("w_multiple", [E], F32, kind="ExternalInput").ap()
    c2q_d = nc.dram_tensor("c2q", [BPC, C, E], F32, kind="ExternalOutput").ap()
    q2c_d = nc.dram_tensor("q2c", [BPC, 1, E], F32, kind="ExternalOutput").ap()

    with (
        tc.tile_pool(name="statics", bufs=1) as statics,
        tc.tile_pool(name="bigio", bufs=2) as bigio,
        tc.tile_pool(name="mid", bufs=2) as mid,
        tc.tile_pool(name="small", bufs=2) as small,
        tc.tile_pool(name="simp", bufs=3) as simp,
        tc.tile_pool(name="c2qp", bufs=4) as c2qp,
        tc.tile_pool(name="trdump", bufs=2, space="PSUM") as trdump,
        tc.tile_pool(name="trip", bufs=2, space="PSUM") as trip,
        tc.tile_pool(name="out2p", bufs=2, space="PSUM") as out2p,
        tc.tile_pool(name="miscp", bufs=2, space="PSUM") as miscp,
    ):
        # ---- one-time constants ----
        ident = statics.tile([128, 128], F32)
        make_identity(nc, ident)
        # ones row: stationary [1, 128] for partition-broadcast matmuls
        ones_row = statics.tile([1, 128], F32)
        nc.vector.memset(ones_row, 1.0)
        # w_question broadcast over the 64 question partitions
        wq_b = statics.tile([Q, E], F32)
        nc.sync.dma_start(
            out=wq_b,
            in_=bass.AP(tensor=wq_d.tensor, offset=wq_d.offset,
                        ap=[[0, Q]] + list(wq_d.ap)),
        )
        # w_multiple as a column per e-chunk: wm_sb[p, eh] = w_m[eh*128+p]
        wm_sb = statics.tile([128, ECH], F32)
        nc.sync.dma_start(out=wm_sb, in_=wm_d.rearrange("(eh p) -> p eh", p=128))

        for b in range(nbatches):
            ctx_r = ctx_d[b].rearrange("(cc p) e -> p cc e", p=128)

            # ---- loads ----
            ctx_sb = bigio.tile([128, CCH, E], F32, tag="ctx_sb")
            q_dup = mid.tile([128, E], F32, tag="q_dup")
            nc.sync.dma_start(out=q_dup[0:Q, :], in_=q_d[b])
            nc.sync.dma_start(out=q_dup[Q:2 * Q, :], in_=q_d[b])

            # ---- q_w row, broadcast to 128 partitions ----
            qw_scr = small.tile([Q, E], F32, tag="qw_scr")
            qw_col = small.tile([Q, 1], F32, tag="qw_col")
            nc.vector.tensor_mul(out=qw_scr, in0=q_dup[0:Q, :], in1=wq_b)
            nc.vector.reduce_sum(out=qw_col, in_=qw_scr, axis=mybir.AxisListType.X)
            qwT_ps = miscp.tile([1, Q], F32, tag="miscp")
            nc.tensor.transpose(qwT_ps, qw_col, ident[0:Q, 0:Q])
            qw_row = small.tile([1, Q], F32, tag="qw_row")
            nc.vector.tensor_copy(out=qw_row, in_=qwT_ps)
            qw_bc_ps = miscp.tile([128, Q], F32, tag="miscp")
            nc.tensor.matmul(qw_bc_ps, lhsT=ones_row, rhs=qw_row,
                             start=True, stop=True)
            qw_bc = small.tile([128, Q], F32, tag="qw_bc")
            nc.vector.tensor_copy(out=qw_bc, in_=qw_bc_ps)

            # ---- qm_aug: [e-part, eh, 65]; cols 0:64 = (q*w_m)T, col 64 = w_c
            qT_ps = miscp.tile([128, ECH, Q], F32, tag="miscp")
            for eh in range(ECH):
                nc.tensor.transpose(
                    qT_ps[:, eh, :], q_dup[0:Q, eh * 128:(eh + 1) * 128],
                    ident[0:Q, 0:Q],
                )
            qm_aug = mid.tile([128, ECH, Q + 1], F32, tag="qm_aug")
            nc.sync.dma_start(
                out=qm_aug[:, :, Q:Q + 1],
                in_=wc_d.rearrange("(eh p) -> p eh", p=128),
            )
            for eh in range(ECH):
                nc.vector.tensor_scalar_mul(
                    out=qm_aug[:, eh, 0:Q], in0=qT_ps[:, eh, :],
                    scalar1=wm_sb[:, eh:eh + 1],
                )

            if _stop("trctx"):
                continue
            # ---- transpose ctx into [e, c] layout ----
            ctxT_sb = bigio.tile([128, ECH, CCH, 128], F32, tag="ctxT_sb")
            for g in range(4):
                nc.sync.dma_start(
                    out=ctx_sb[:, 4 * g:4 * (g + 1), :],
                    in_=ctx_r[:, 4 * g:4 * (g + 1), :],
                )
                for eh in range(ECH):
                    ctxT_ps = trdump.tile([128, 4, 128], F32, tag="trdump")
                    for k in range(4):
                        cc = 4 * g + k
                        nc.tensor.transpose(
                            ctxT_ps[:, k, :],
                            ctx_sb[:, cc, eh * 128:(eh + 1) * 128],
                            ident,
                        )
                    nc.scalar.copy(
                        out=ctxT_sb[:, eh, 4 * g:4 * (g + 1), :], in_=ctxT_ps
                    )

            if _stop("mm1"):
                continue
            # ---- mm1 + row softmax per c-chunk ----
            negm = small.tile([128, CCH], F32, tag="negm")
            m_sb = small.tile([128, CCH], F32, tag="m_sb")
            s_sb = small.tile([128, CCH], F32, tag="s_sb")
            r_sb = small.tile([128, CCH], F32, tag="r_sb")
            e_sb = mid.tile([128, CCH, Q], F32, tag="e_sb")
            for cc in range(CCH):
                tri = trip.tile([128, Q + 1], F32, tag="trip")
                for eh in range(ECH):
                    nc.tensor.matmul(
                        tri, lhsT=ctxT_sb[:, eh, cc, :], rhs=qm_aug[:, eh, :],
                        start=(eh == 0), stop=(eh == ECH - 1),
                    )
                if _stop("soft"):
                    continue
                simn = simp.tile([128, Q], F32, tag="simn")
                # simn = tri + q_w; negm[:,cc] = -max_j simn
                nc.vector.tensor_add(out=simn, in0=tri[:, 0:Q], in1=qw_bc)
                nc.vector.reduce_max(
                    out=negm[:, cc:cc + 1], in_=simn,
                    axis=mybir.AxisListType.X, negate=True,
                )
                # m = c_w - negm = c_w + max_j sim'
                nc.vector.tensor_sub(
                    out=m_sb[:, cc:cc + 1], in0=tri[:, Q:Q + 1],
                    in1=negm[:, cc:cc + 1],
                )
                # e = exp(sim' - max) = exp(simn + negm)
                nc.scalar.activation(
                    out=e_sb[:, cc, :], in_=simn, func=ACTF.Exp,
                    bias=negm[:, cc:cc + 1], scale=1.0,
                )
                nc.vector.reduce_sum(
                    out=s_sb[:, cc:cc + 1], in_=e_sb[:, cc, :],
                    axis=mybir.AxisListType.X,
                )
                nc.vector.reciprocal(
                    out=r_sb[:, cc:cc + 1], in_=s_sb[:, cc:cc + 1]
                )

            if _stop("tre"):
                continue
            # ---- transpose e into [j, c] layout (pairs of chunks) ----
            eT_sb = mid.tile([128, CCH // 2, 128], F32, tag="eT_sb")
            for tg in range(2):
                eT_ps = trdump.tile([128, 4, 128], F32, tag="trdump")
                for k in range(4):
                    tt = 4 * tg + k
                    nc.tensor.transpose(
                        eT_ps[:, k, :], e_sb[:, 2 * tt:2 * tt + 2, :], ident
                    )
                nc.scalar.copy(
                    out=eT_sb[:, 4 * tg:4 * (tg + 1), :], in_=eT_ps
                )

            if _stop("mm2"):
                continue
            # ---- mm2: c2q chunks; normalize on evacuation ----
            for cc in range(CCH):
                tt, half = cc // 2, cc % 2
                out2 = out2p.tile([128, E], F32, tag="out2p")
                nc.tensor.matmul(
                    out2,
                    lhsT=eT_sb[64 * half:64 * (half + 1), tt, :],
                    rhs=q_dup[64 * half:64 * (half + 1), :],
                    start=True, stop=True,
                )
                c2q_t = c2qp.tile([128, E], F32, tag="c2q_t")
                nc.vector.tensor_scalar_mul(
                    out=c2q_t, in0=out2, scalar1=r_sb[:, cc:cc + 1]
                )
                nc.sync.dma_start(
                    out=c2q_d[b, 128 * cc:128 * (cc + 1), :], in_=c2q_t
                )

            if _stop("q2c"):
                continue
            # ---- q2c: softmax over all 2048 rows of m, then beta @ ctx ----
            rm = small.tile([128, 1], F32, tag="rm")
            nc.vector.reduce_max(out=rm, in_=m_sb, axis=mybir.AxisListType.X)
            rmT_ps = miscp.tile([1, 128], F32, tag="miscp")
            nc.tensor.transpose(rmT_ps, rm, ident)
            ng = small.tile([1, 1], F32, tag="ng")
            nc.vector.reduce_max(
                out=ng, in_=rmT_ps, axis=mybir.AxisListType.X, negate=True
            )
            ngbc_ps = miscp.tile([128, 1], F32, tag="miscp")
            nc.tensor.matmul(ngbc_ps, lhsT=ones_row, rhs=ng,
                             start=True, stop=True)
            ng_b = small.tile([128, 1], F32, tag="ng_b")
            nc.vector.tensor_copy(out=ng_b, in_=ngbc_ps)
            be = small.tile([128, CCH], F32, tag="be")
            bsum = small.tile([128, 1], F32, tag="bsum")
            nc.scalar.activation(
                out=be, in_=m_sb, func=ACTF.Exp, bias=ng_b[:, 0:1], scale=1.0,
                accum_out=bsum,
            )
            bsT_ps = miscp.tile([1, 128], F32, tag="miscp")
            nc.tensor.transpose(bsT_ps, bsum, ident)
            gs = small.tile([1, 1], F32, tag="gs")
            nc.vector.reduce_sum(out=gs, in_=bsT_ps, axis=mybir.AxisListType.X)
            rg = small.tile([1, 1], F32, tag="rg")
            nc.vector.reciprocal(out=rg, in_=gs)
            rgbc_ps = miscp.tile([128, 1], F32, tag="miscp")
            nc.tensor.matmul(rgbc_ps, lhsT=ones_row, rhs=rg,
                             start=True, stop=True)
            rg_b = small.tile([128, 1], F32, tag="rg_b")
            nc.vector.tensor_copy(out=rg_b, in_=rgbc_ps)

            out3 = miscp.tile([128, ECH], F32, tag="miscp")
            for eh in range(ECH):
                for cc in range(CCH):
                    nc.tensor.matmul(
                        out3[:, eh:eh + 1],
                        lhsT=ctx_sb[:, cc, eh * 128:(eh + 1) * 128],
                        rhs=be[:, cc:cc + 1],
                        start=(cc == 0), stop=(cc == CCH - 1),
                    )
            q2cT = small.tile([128, ECH], F32, tag="q2cT")
            nc.vector.tensor_scalar_mul(out=q2cT, in0=out3, scalar1=rg_b[:, 0:1])
            nc.sync.dma_start(
                out=q2c_d[b, 0, :].rearrange("(eh p) -> p eh", p=128), in_=q2cT
            )


def build(nbatches=BPC, stop_after=None):
    from concourse import bacc

    nc = bacc.Bacc("TRN2", target_bir_lowering=False, debug=False)
    with tile.TileContext(nc) as tc:
        _emit(tc, nbatches=nbatches, stop_after=stop_after)
    nc.compile()
    return nc


_nc = None


def _get_nc():
    global _nc
    if _nc is None:
        _nc = build()
    return _nc


def make_in_maps(context, question, w_question, w_context, w_multiple):
    context = np.ascontiguousarray(np.asarray(context, dtype=np.float32))
    question = np.ascontiguousarray(np.asarray(question, dtype=np.float32))
    w_question = np.ascontiguousarray(np.asarray(w_question, dtype=np.float32))
    w_context = np.ascontiguousarray(np.asarray(w_context, dtype=np.float32))
    w_multiple = np.ascontiguousarray(np.asarray(w_multiple, dtype=np.float32))
    return [
        {
            "context": context[k * BPC:(k + 1) * BPC],
            "question": question[k * BPC:(k + 1) * BPC],
            "w_question": w_question,
            "w_context": w_context,
            "w_multiple": w_multiple,
        }
        for k in range(NCORES)
    ]


def assemble(results):
    c2q = np.concatenate([results[k]["c2q"] for k in range(NCORES)], axis=0)
    q2c = np.concatenate([results[k]["q2c"] for k in range(NCORES)], axis=0)
    return c2q, q2c


def kernel(**inputs):
    nc = _get_nc()
    in_maps = make_in_maps(**inputs)
    res = run_bass_kernel_spmd(nc, in_maps, list(range(NCORES)))
    return assemble(res.results)
